# revision 12
# baseline (speedup 1.0000x reference)
"""Trainium2 Bass kernel for the non-local attention block (nn_CPP_80676665688885).

Sharding: pure data-parallel over batch — 1 sample per NeuronCore (B=8, 8 cores).
BatchNorm batch-statistics are combined with a tiny (2 KB) AllReduce.

Per-core algorithm (sample x: (C=256, N=4096) with N = 64x64 spatial):
  theta = Wt@x + bt            (CI=128, N)     TensorE + DVE bias-copy
  phi   = maxpool2(Wp@x + bp)  (CI, M=1024)
  g     = maxpool2(Wg@x + bg)  (CI, M) -> transposed to gT (M, CI)
  fT    = phi^T @ theta        (M, N) tiles    computed m-chunk-wise, exp on ScalarE
          (no max-subtraction: |f| < 48 < 88 so fp32 exp is exact-safe)
  y     = g @ softmax-weights: yT (CI, N) accumulated over m-chunks on TensorE
  s[n]  = sum_m exp(fT[m,n])   via ones-vector matmuls accumulated in PSUM
  y_n   = y * (1/s)            1/s computed exactly on (128,x) layout via DRAM bounce
  wy    = Ww @ y_n             (C, N); conv bias bw dropped (cancels in BatchNorm)
  S1,S2 = per-channel sum / sum-of-squares -> AllReduce over 8 cores
  z     = (wy - mean)*rsqrt(var+eps)*gamma + beta + x ; out = max_n z   (C,)
"""

import numpy as np
from contextlib import ExitStack

import concourse.bass as bass
import concourse.bacc as bacc
import concourse.tile as tile
from concourse import mybir
from concourse.bass_utils import run_bass_kernel_spmd

F32 = mybir.dt.float32
AF = mybir.ActivationFunctionType
ALU = mybir.AluOpType
AX = mybir.AxisListType

B = 8
C = 256
CI = 128
N = 4096          # 64*64
M = 1024          # 32*32 after 2x2 maxpool
NT = 512          # n-tile (PSUM bank width in fp32)
NTILES = N // NT  # 8
MCH = M // 128    # 8 m-chunks
CCH = C // 128    # 2 channel chunks
EPS = 1e-5
INV_CNT = 1.0 / (B * N)

_CACHE = {}


def _build():
    nc = bacc.Bacc("TRN2", num_devices=B)

    x_d = nc.declare_dram_parameter("x", [C, N], F32, False)
    wtT_d = nc.declare_dram_parameter("WtT", [C, CI], F32, False)
    wpT_d = nc.declare_dram_parameter("WpT", [C, CI], F32, False)
    wgT_d = nc.declare_dram_parameter("WgT", [C, CI], F32, False)
    wwT_d = nc.declare_dram_parameter("WwT", [CI, C], F32, False)
    bt_d = nc.declare_dram_parameter("bt", [CI, 1], F32, False)
    bp_d = nc.declare_dram_parameter("bp", [CI, 1], F32, False)
    bg_d = nc.declare_dram_parameter("bg", [CI, 1], F32, False)
    gamma_d = nc.declare_dram_parameter("gamma", [128, CCH], F32, False)
    beta_d = nc.declare_dram_parameter("beta", [128, CCH], F32, False)
    out_d = nc.declare_dram_parameter("out", [CCH, 128], F32, True)

    ident_d = nc.inline_tensor(np.eye(128, dtype=np.float32), name="ident")

    # DRAM bounce buffers
    s_dram = nc.dram_tensor("s_bounce", [1, N], F32)
    r_dram = nc.dram_tensor("r_bounce", [1, N], F32)
    stats_in = nc.dram_tensor("stats_in", [128, 2 * CCH], F32)
    stats_out = nc.dram_tensor("stats_out", [128, 2 * CCH], F32, addr_space="Shared")

    with ExitStack() as ctx:
        tc = ctx.enter_context(tile.TileContext(nc))
        consts = ctx.enter_context(tc.tile_pool(name="consts", bufs=1))
        persist = ctx.enter_context(tc.tile_pool(name="persist", bufs=1))
        scratch = ctx.enter_context(tc.tile_pool(name="scratch", bufs=2))
        efp = ctx.enter_context(tc.tile_pool(name="efp", bufs=4))
        small = ctx.enter_context(tc.tile_pool(name="small", bufs=4))
        ps_ft = ctx.enter_context(tc.tile_pool(name="ps_ft", bufs=2, space="PSUM"))
        ps_y = ctx.enter_context(tc.tile_pool(name="ps_y", bufs=2, space="PSUM"))
        ps_s = ctx.enter_context(tc.tile_pool(name="ps_s", bufs=1, space="PSUM"))
        ps_rb = ctx.enter_context(tc.tile_pool(name="ps_rb", bufs=1, space="PSUM"))
        ps_cv = ctx.enter_context(tc.tile_pool(name="ps_cv", bufs=2, space="PSUM"))

        # ---- constants / weights into SBUF ----
        ident = consts.tile([128, 128], F32)
        nc.sync.dma_start(out=ident, in_=ident_d[:, :])
        ones_k = consts.tile([128, 1], F32)
        nc.vector.memset(ones_k, 1.0)
        ones_p = consts.tile([1, 128], F32)
        nc.vector.memset(ones_p, 1.0)
        eps_sb = consts.tile([128, 1], F32)
        nc.vector.memset(eps_sb, EPS)

        wt_sb = consts.tile([128, CCH, CI], F32)
        wp_sb = consts.tile([128, CCH, CI], F32)
        wg_sb = consts.tile([128, CCH, CI], F32)
        for ch in range(CCH):
            nc.sync.dma_start(out=wt_sb[:, ch, :], in_=wtT_d[ch * 128:(ch + 1) * 128, :])
            nc.sync.dma_start(out=wp_sb[:, ch, :], in_=wpT_d[ch * 128:(ch + 1) * 128, :])
            nc.sync.dma_start(out=wg_sb[:, ch, :], in_=wgT_d[ch * 128:(ch + 1) * 128, :])
        ww_sb = consts.tile([128, CCH, 128], F32)
        for ch in range(CCH):
            nc.sync.dma_start(out=ww_sb[:, ch, :], in_=wwT_d[:, ch * 128:(ch + 1) * 128])
        bt_sb = consts.tile([128, 1], F32)
        bp_sb = consts.tile([128, 1], F32)
        bg_sb = consts.tile([128, 1], F32)
        nc.sync.dma_start(out=bt_sb, in_=bt_d[:, :])
        nc.sync.dma_start(out=bp_sb, in_=bp_d[:, :])
        nc.sync.dma_start(out=bg_sb, in_=bg_d[:, :])
        gamma_sb = consts.tile([128, CCH], F32)
        beta_sb = consts.tile([128, CCH], F32)
        nc.sync.dma_start(out=gamma_sb, in_=gamma_d[:, :])
        nc.sync.dma_start(out=beta_sb, in_=beta_d[:, :])

        # ---- x into SBUF ----
        x_sb = [persist.tile([128, N], F32, tag=f"x{ch}", name=f"x_sb{ch}") for ch in range(CCH)]
        for ch in range(CCH):
            nc.sync.dma_start(out=x_sb[ch], in_=x_d[ch * 128:(ch + 1) * 128, :])

        # ---- projections: theta (kept), phi/g (pooled) ----
        theta = persist.tile([128, N], F32, tag="theta")
        phi_full = scratch.tile([128, N], F32, tag="s4")
        g_full = scratch.tile([128, N], F32, tag="s4")

        def conv(dst, w_sb, b_sb):
            for it in range(NTILES):
                ps = ps_cv.tile([128, NT], F32, tag="cv")
                for ch in range(CCH):
                    nc.tensor.matmul(
                        ps, lhsT=w_sb[:, ch, :],
                        rhs=x_sb[ch][:, it * NT:(it + 1) * NT],
                        start=(ch == 0), stop=(ch == CCH - 1))
                nc.vector.tensor_scalar_add(
                    out=dst[:, it * NT:(it + 1) * NT], in0=ps, scalar1=b_sb)

        conv(theta, wt_sb, bt_sb)
        conv(phi_full, wp_sb, bp_sb)
        conv(g_full, wg_sb, bg_sb)

        # ---- 2x2 maxpool on phi and g (GpSimd tensor_tensor max on strided views) ----
        phi_pool = persist.tile([128, M], F32, tag="phip")
        g_pool = persist.tile([128, M], F32, tag="gp")
        pp1 = scratch.tile([128, 64 * 32], F32, tag="pool1")
        gp1 = scratch.tile([128, 64 * 32], F32, tag="pool1")
        for src, mid, dst in ((phi_full, pp1, phi_pool), (g_full, gp1, g_pool)):
            sr = src.rearrange("p (h wp t) -> p h wp t", h=64, wp=32, t=2)
            nc.vector.tensor_tensor(
                out=mid.rearrange("p (h wp) -> p h wp", h=64),
                in0=sr[:, :, :, 0], in1=sr[:, :, :, 1], op=ALU.max)
            mr = mid.rearrange("p (hp s wp) -> p hp s wp", hp=32, s=2, wp=32)
            nc.vector.tensor_tensor(
                out=dst.rearrange("p (hp wp) -> p hp wp", hp=32),
                in0=mr[:, :, 0, :], in1=mr[:, :, 1, :], op=ALU.max)

        # ---- transpose g_pool (CI, M) -> gT chunks (m=128, CI) ----
        gT = persist.tile([128, MCH, CI], F32, tag="gT")
        for mc in range(MCH):
            tp = ps_cv.tile([128, 128], F32, tag="cv")
            nc.tensor.transpose(tp, g_pool[:, mc * 128:(mc + 1) * 128], ident)
            nc.scalar.copy(out=gT[:, mc, :], in_=tp)

        # ---- attention + normalization + W-conv, per n-tile ----
        y_n = persist.tile([128, N], F32, tag="yn")
        wy = [scratch.tile([128, N], F32, tag="s4", name=f"wy{ch}") for ch in range(CCH)]
        s_all = persist.tile([1, N], F32, tag="sall")
        rT = persist.tile([128, NTILES * (NT // 128)], F32, tag="rT")
        r_flat = persist.tile([1, N], F32, tag="rflat")
        s1p = persist.tile([128, CCH, NTILES], F32, tag="s1p")
        s2p = persist.tile([128, CCH, NTILES], F32, tag="s2p")

        for it in range(NTILES):
            sl = slice(it * NT, (it + 1) * NT)
            th_t = theta[:, sl]
            yps = ps_y.tile([128, NT], F32, tag="yps")
            sps = ps_s.tile([1, NT], F32, tag="sps")
            for mc in range(MCH):
                fps = ps_ft.tile([128, NT], F32, tag="ft")
                nc.tensor.matmul(fps, lhsT=phi_pool[:, mc * 128:(mc + 1) * 128],
                                 rhs=th_t, start=True, stop=True)
                ef = efp.tile([128, NT], F32, tag="ef")
                nc.scalar.activation(out=ef, in_=fps, func=AF.Exp)
                nc.tensor.matmul(yps, lhsT=gT[:, mc, :], rhs=ef,
                                 start=(mc == 0), stop=(mc == MCH - 1))
                nc.tensor.matmul(sps, lhsT=ones_k, rhs=ef,
                                 start=(mc == 0), stop=(mc == MCH - 1))

            # s -> SBUF, bounce via DRAM into (128, NT/128) layout, reciprocal, back
            nc.scalar.copy(out=s_all[:, sl], in_=sps)
            nc.sync.dma_start(out=s_dram[:, sl], in_=s_all[:, sl])
            f4 = NT // 128
            sl4 = slice(it * f4, (it + 1) * f4)
            sT_t = small.tile([128, f4], F32, tag="sT")
            nc.sync.dma_start(out=sT_t, in_=s_dram[0, sl].rearrange("(p f) -> p f", p=128))
            nc.vector.reciprocal(out=rT[:, sl4], in_=sT_t)
            nc.sync.dma_start(out=r_dram[0, sl].rearrange("(p f) -> p f", p=128), in_=rT[:, sl4])
            nc.sync.dma_start(out=r_flat[:, sl], in_=r_dram[:, sl])

            # broadcast r across partitions via K=1 matmul, then normalize y
            rbps = ps_rb.tile([128, NT], F32, tag="rb")
            nc.tensor.matmul(rbps, lhsT=ones_p, rhs=r_flat[:, sl], start=True, stop=True)
            rb_sb = small.tile([128, NT], F32, tag="rb_sb")
            nc.vector.tensor_copy(out=rb_sb, in_=rbps)
            nc.vector.scalar_tensor_tensor(
                out=y_n[:, sl], in0=yps, scalar=1.0, in1=rb_sb,
                op0=ALU.mult, op1=ALU.mult)

            # W conv on normalized y; accumulate BN partial stats
            for ch in range(CCH):
                wps = ps_cv.tile([128, NT], F32, tag="cv")
                nc.tensor.matmul(wps, lhsT=ww_sb[:, ch, :], rhs=y_n[:, sl],
                                 start=True, stop=True)
                nc.vector.tensor_scalar(
                    out=wy[ch][:, sl], in0=wps, scalar1=0.0, scalar2=None,
                    op0=ALU.add, op1=ALU.add, accum_out=s1p[:, ch, it:it + 1])
                sqt = efp.tile([128, NT], F32, tag="sqtrash")
                nc.scalar.activation(
                    out=sqt, in_=wy[ch][:, sl], func=AF.Square,
                    accum_out=s2p[:, ch, it:it + 1])

        # ---- combine partials, AllReduce, finalize ----
        stats_sb = small.tile([128, 2 * CCH], F32, tag="stats")
        for ch in range(CCH):
            nc.vector.tensor_reduce(out=stats_sb[:, 2 * ch:2 * ch + 1],
                                    in_=s1p[:, ch, :], axis=AX.X, op=ALU.add)
            nc.vector.tensor_reduce(out=stats_sb[:, 2 * ch + 1:2 * ch + 2],
                                    in_=s2p[:, ch, :], axis=AX.X, op=ALU.add)
        nc.sync.dma_start(out=stats_in[:, :], in_=stats_sb)
        nc.gpsimd.collective_compute(
            "AllReduce", ALU.add, replica_groups=[list(range(B))],
            ins=[stats_in[:, :]], outs=[stats_out[:, :]])
        stats_g = small.tile([128, 2 * CCH], F32, tag="statsg")
        nc.sync.dma_start(out=stats_g, in_=stats_out[:, :])

        out_sb = small.tile([128, CCH], F32, tag="outsb")
        for ch in range(CCH):
            mean = small.tile([128, 1], F32, tag="fin")
            e2 = small.tile([128, 1], F32, tag="fin")
            m2 = small.tile([128, 1], F32, tag="fin")
            var = small.tile([128, 1], F32, tag="fin")
            nc.vector.tensor_scalar_mul(out=mean, in0=stats_g[:, 2 * ch:2 * ch + 1], scalar1=INV_CNT)
            nc.vector.tensor_scalar_mul(out=e2, in0=stats_g[:, 2 * ch + 1:2 * ch + 2], scalar1=INV_CNT)
            nc.scalar.square(out=m2, in_=mean)
            nc.vector.tensor_tensor(out=var, in0=e2, in1=m2, op=ALU.subtract)
            sd = small.tile([128, 1], F32, tag="fin")
            nc.scalar.activation(out=sd, in_=var, func=AF.Sqrt, bias=eps_sb, scale=1.0)
            inv = small.tile([128, 1], F32, tag="fin")
            nc.vector.reciprocal(out=inv, in_=sd)
            scale = small.tile([128, 1], F32, tag="fin")
            nc.vector.tensor_tensor(out=scale, in0=inv, in1=gamma_sb[:, ch:ch + 1], op=ALU.mult)
            negshift = small.tile([128, 1], F32, tag="fin")
            nc.vector.scalar_tensor_tensor(
                out=negshift, in0=mean, scalar=scale, in1=beta_sb[:, ch:ch + 1],
                op0=ALU.mult, op1=ALU.subtract)
            # z' = wy*scale + x  (in place over wy); out = max_n z' - negshift
            nc.vector.scalar_tensor_tensor(
                out=wy[ch][:, :], in0=wy[ch][:, :], scalar=scale, in1=x_sb[ch],
                op0=ALU.mult, op1=ALU.add)
            mx = small.tile([128, 1], F32, tag="fin")
            nc.vector.tensor_reduce(out=mx, in_=wy[ch][:, :], axis=AX.X, op=ALU.max)
            nc.vector.tensor_tensor(out=out_sb[:, ch:ch + 1], in0=mx, in1=negshift, op=ALU.subtract)
        for ch in range(CCH):
            nc.sync.dma_start(out=out_d[ch, :].rearrange("(p one) -> p one", one=1),
                              in_=out_sb[:, ch:ch + 1])

    nc.compile()
    return nc


_LAST = {}


def kernel(**inputs):
    x = np.ascontiguousarray(inputs["x"], dtype=np.float32)      # (8, 256, 64, 64)
    Wg = np.asarray(inputs["Wg"], dtype=np.float32)
    bg = np.asarray(inputs["bg"], dtype=np.float32)
    Wt = np.asarray(inputs["Wt"], dtype=np.float32)
    bt = np.asarray(inputs["bt"], dtype=np.float32)
    Wp = np.asarray(inputs["Wp"], dtype=np.float32)
    bp = np.asarray(inputs["bp"], dtype=np.float32)
    Ww = np.asarray(inputs["Ww"], dtype=np.float32)
    gamma = np.asarray(inputs["gamma"], dtype=np.float32)
    beta = np.asarray(inputs["beta"], dtype=np.float32)

    if "nc" not in _CACHE:
        _CACHE["nc"] = _build()
    nc = _CACHE["nc"]

    shared = {
        "WtT": np.ascontiguousarray(Wt.T),
        "WpT": np.ascontiguousarray(Wp.T),
        "WgT": np.ascontiguousarray(Wg.T),
        "WwT": np.ascontiguousarray(Ww.T),
        "bt": np.ascontiguousarray(bt.reshape(CI, 1)),
        "bp": np.ascontiguousarray(bp.reshape(CI, 1)),
        "bg": np.ascontiguousarray(bg.reshape(CI, 1)),
        "gamma": np.ascontiguousarray(gamma.reshape(CCH, 128).T),
        "beta": np.ascontiguousarray(beta.reshape(CCH, 128).T),
    }
    in_maps = [dict(shared, x=np.ascontiguousarray(x[b].reshape(C, N)))
               for b in range(B)]
    import os
    trace = bool(int(os.environ.get("KERNEL_TRACE", "0")))
    res = run_bass_kernel_spmd(nc, in_maps, core_ids=list(range(B)), trace=trace)
    _LAST["res"] = res
    out = np.stack([np.asarray(res.results[b]["out"]).reshape(C) for b in range(B)])
    return out.reshape(B, C, 1, 1).astype(np.float32)


if __name__ == "__main__":
    pass


# revision 16
# speedup vs baseline: 1.3021x; 1.3021x over previous
"""Trainium2 Bass kernel for the non-local attention block (nn_CPP_80676665688885).

Sharding: pure data-parallel over batch — 1 sample per NeuronCore (B=8, 8 cores).
BatchNorm batch-statistics are combined with a tiny (2 KB) AllGather.

fp32 matmuls on TRN2 run in LOW_HIGH mode (2 passes, ~2.5 cyc/col) — ~5x the
cost of bf16. So every large matmul here is decomposed into bf16 passes:
  exact-ish (error ~2^-16): A@B = A_hi@B_hi + A_hi@B_lo + A_lo@B_hi
  where X_hi = bf16(X), X_lo = bf16(X - X_hi); fp32 accumulation in PSUM.
exp(fT) is written directly as bf16: its quantization acts as a correlated
perturbation of softmax logits (numerator and denominator use the same
values), so the final error stays ~1e-4 relative.

Per-core algorithm (sample x: (C=256, N=4096), N = 64x64 spatial):
  theta = Wt@x + bt  (split hi/lo)     phi,g = maxpool2(conv)  (phi split, g
  transposed then split)
  fT    = phi^T @ theta  3 bf16 passes; exp on ScalarE -> expf bf16
  y     = gT^T @ expf    2 bf16 passes (gT hi/lo), accumulated over m-chunks
  s[n]  = ones^T @ expf  1 bf16 pass,  accumulated over m-chunks
  y_n   = y * (1/s)  (reciprocal exactly on (128,x) layout via DRAM bounce)
  wy    = Ww @ y_n   (native fp32; bias bw dropped — cancels in BatchNorm)
  S1,S2 per channel -> AllGather over 8 cores -> local sum
  z     = (wy - mean)*rsqrt(var+eps)*gamma + beta + x ; out = max_n z
"""

import numpy as np
from contextlib import ExitStack

import concourse.bass as bass
import concourse.bacc as bacc
import concourse.tile as tile
from concourse import mybir
from concourse.bass_utils import run_bass_kernel_spmd

F32 = mybir.dt.float32
BF16 = mybir.dt.bfloat16
AF = mybir.ActivationFunctionType
ALU = mybir.AluOpType
AX = mybir.AxisListType

B = 8
C = 256
CI = 128
N = 4096          # 64*64
M = 1024          # 32*32 after 2x2 maxpool
NT = 512          # n-tile (PSUM bank width in fp32)
NTILES = N // NT  # 8
MCH = M // 128    # 8 m-chunks
CCH = C // 128    # 2 channel chunks
EPS = 1e-5
INV_CNT = 1.0 / (B * N)

_CACHE = {}


def _build():
    nc = bacc.Bacc("TRN2", num_devices=B)

    x_d = nc.declare_dram_parameter("x", [C, N], F32, False)
    # hi/lo bf16-split projection weights, pre-transposed host-side
    w_hi_d = {}
    w_lo_d = {}
    for nm in ("t", "p", "g"):
        w_hi_d[nm] = nc.declare_dram_parameter(f"W{nm}Thi", [C, CI], BF16, False)
        w_lo_d[nm] = nc.declare_dram_parameter(f"W{nm}Tlo", [C, CI], BF16, False)
    wwT_d = nc.declare_dram_parameter("WwT", [CI, C], F32, False)
    bt_d = nc.declare_dram_parameter("bt", [CI, 1], F32, False)
    bp_d = nc.declare_dram_parameter("bp", [CI, 1], F32, False)
    bg_d = nc.declare_dram_parameter("bg", [CI, 1], F32, False)
    gamma_d = nc.declare_dram_parameter("gamma", [128, CCH], F32, False)
    beta_d = nc.declare_dram_parameter("beta", [128, CCH], F32, False)
    out_d = nc.declare_dram_parameter("out", [CCH, 128], F32, True)

    ident_d = nc.inline_tensor(np.eye(128, dtype=np.float32), name="ident")

    # DRAM bounce buffers
    s_dram = nc.dram_tensor("s_bounce", [1, N], F32)
    r_dram = nc.dram_tensor("r_bounce", [1, N], F32)
    stats_in = nc.dram_tensor("stats_in", [128, 2 * CCH], F32)
    stats_out = nc.dram_tensor("stats_out", [128, 2 * CCH], F32,
                               addr_space="Shared")

    with ExitStack() as ctx:
        tc = ctx.enter_context(tile.TileContext(nc))
        consts = ctx.enter_context(tc.tile_pool(name="consts", bufs=1))
        persist = ctx.enter_context(tc.tile_pool(name="persist", bufs=1))
        scratch = ctx.enter_context(tc.tile_pool(name="scratch", bufs=2))
        efp = ctx.enter_context(tc.tile_pool(name="efp", bufs=5))
        small = ctx.enter_context(tc.tile_pool(name="small", bufs=4))
        ps_ft = ctx.enter_context(tc.tile_pool(name="ps_ft", bufs=2, space="PSUM"))
        ps_y = ctx.enter_context(tc.tile_pool(name="ps_y", bufs=2, space="PSUM"))
        ps_s = ctx.enter_context(tc.tile_pool(name="ps_s", bufs=1, space="PSUM"))
        ps_rb = ctx.enter_context(tc.tile_pool(name="ps_rb", bufs=1, space="PSUM"))
        ps_cv = ctx.enter_context(tc.tile_pool(name="ps_cv", bufs=2, space="PSUM"))

        # ---- constants / weights into SBUF ----
        ident = consts.tile([128, 128], F32)
        nc.sync.dma_start(out=ident, in_=ident_d[:, :])
        ones_k = consts.tile([128, 1], BF16)
        nc.vector.memset(ones_k, 1.0)
        ones_p = consts.tile([1, 128], F32)
        nc.vector.memset(ones_p, 1.0)
        eps_sb = consts.tile([128, 1], F32)
        nc.vector.memset(eps_sb, EPS)

        w_hi = {}
        w_lo = {}
        for nm in ("t", "p", "g"):
            w_hi[nm] = consts.tile([128, CCH, CI], BF16, name=f"w_hi_{nm}")
            w_lo[nm] = consts.tile([128, CCH, CI], BF16, name=f"w_lo_{nm}")
            for ch in range(CCH):
                cs = slice(ch * 128, (ch + 1) * 128)
                nc.sync.dma_start(out=w_hi[nm][:, ch, :], in_=w_hi_d[nm][cs, :])
                nc.sync.dma_start(out=w_lo[nm][:, ch, :], in_=w_lo_d[nm][cs, :])
        ww_sb = consts.tile([128, CCH, 128], F32)
        for ch in range(CCH):
            nc.sync.dma_start(out=ww_sb[:, ch, :], in_=wwT_d[:, ch * 128:(ch + 1) * 128])
        bt_sb = consts.tile([128, 1], F32)
        bp_sb = consts.tile([128, 1], F32)
        bg_sb = consts.tile([128, 1], F32)
        nc.sync.dma_start(out=bt_sb, in_=bt_d[:, :])
        nc.sync.dma_start(out=bp_sb, in_=bp_d[:, :])
        nc.sync.dma_start(out=bg_sb, in_=bg_d[:, :])
        gamma_sb = consts.tile([128, CCH], F32)
        beta_sb = consts.tile([128, CCH], F32)
        nc.sync.dma_start(out=gamma_sb, in_=gamma_d[:, :])
        nc.sync.dma_start(out=beta_sb, in_=beta_d[:, :])

        # ---- x into SBUF, split hi/lo ----
        x_sb = [persist.tile([128, N], F32, tag=f"x{ch}", name=f"x_sb{ch}")
                for ch in range(CCH)]
        x_hi = [persist.tile([128, N], BF16, tag=f"xh{ch}", name=f"x_hi{ch}")
                for ch in range(CCH)]
        x_lo = [persist.tile([128, N], BF16, tag=f"xl{ch}", name=f"x_lo{ch}")
                for ch in range(CCH)]
        for ch in range(CCH):
            nc.sync.dma_start(out=x_sb[ch], in_=x_d[ch * 128:(ch + 1) * 128, :])
            nc.scalar.copy(out=x_hi[ch], in_=x_sb[ch])
            nc.vector.tensor_tensor(out=x_lo[ch], in0=x_sb[ch], in1=x_hi[ch],
                                    op=ALU.subtract)

        # ---- projections (3-term bf16 conv) ----
        # theta: kept as hi/lo bf16 tiles; phi/g: fp32 for pooling
        th_hi = persist.tile([128, N], BF16, tag="thh")
        th_lo = persist.tile([128, N], BF16, tag="thl")
        phi_full = scratch.tile([128, N], F32, tag="s4")
        g_full = scratch.tile([128, N], F32, tag="s4")

        def conv_mms(ps, nm, sl):
            terms = ((w_hi[nm], x_hi), (w_hi[nm], x_lo), (w_lo[nm], x_hi))
            nterm = len(terms) * CCH
            k = 0
            for ch in range(CCH):
                for lhs, rhs in terms:
                    nc.tensor.matmul(ps, lhsT=lhs[:, ch, :], rhs=rhs[ch][:, sl],
                                     start=(k == 0), stop=(k == nterm - 1))
                    k += 1

        for it in range(NTILES):
            sl = slice(it * NT, (it + 1) * NT)
            ps = ps_cv.tile([128, NT], F32, tag="cv")
            conv_mms(ps, "t", sl)
            # theta + bias, split hi/lo (hi on ScalarE, lo on VectorE)
            nc.scalar.activation(out=th_hi[:, sl], in_=ps, func=AF.Identity,
                                 bias=bt_sb, scale=1.0)
            nc.vector.scalar_tensor_tensor(out=th_lo[:, sl], in0=ps, scalar=bt_sb,
                                           in1=th_hi[:, sl], op0=ALU.add,
                                           op1=ALU.subtract)
        for dst, nm, b_sb in ((phi_full, "p", bp_sb), (g_full, "g", bg_sb)):
            for it in range(NTILES):
                sl = slice(it * NT, (it + 1) * NT)
                ps = ps_cv.tile([128, NT], F32, tag="cv")
                conv_mms(ps, nm, sl)
                nc.vector.tensor_scalar_add(out=dst[:, sl], in0=ps, scalar1=b_sb)

        # ---- 2x2 maxpool on phi and g ----
        phi_pool = persist.tile([128, M], F32, tag="phip")
        g_pool = persist.tile([128, M], F32, tag="gp")
        pp1 = scratch.tile([128, 64 * 32], F32, tag="pool1")
        gp1 = scratch.tile([128, 64 * 32], F32, tag="pool1")
        for src, mid, dst in ((phi_full, pp1, phi_pool), (g_full, gp1, g_pool)):
            sr = src.rearrange("p (h wp t) -> p h wp t", h=64, wp=32, t=2)
            nc.vector.tensor_tensor(
                out=mid.rearrange("p (h wp) -> p h wp", h=64),
                in0=sr[:, :, :, 0], in1=sr[:, :, :, 1], op=ALU.max)
            mr = mid.rearrange("p (hp s wp) -> p hp s wp", hp=32, s=2, wp=32)
            nc.vector.tensor_tensor(
                out=dst.rearrange("p (hp wp) -> p hp wp", hp=32),
                in0=mr[:, :, 0, :], in1=mr[:, :, 1, :], op=ALU.max)

        # phi hi/lo split
        phi_hi = persist.tile([128, M], BF16, tag="phih")
        phi_lo = persist.tile([128, M], BF16, tag="phil")
        nc.scalar.copy(out=phi_hi, in_=phi_pool)
        nc.vector.tensor_tensor(out=phi_lo, in0=phi_pool, in1=phi_hi,
                                op=ALU.subtract)

        # ---- transpose g_pool (CI, M) -> gT chunks (m=128, CI), split hi/lo ----
        gT32 = persist.tile([128, MCH, CI], F32, tag="gT32")
        gT_hi = persist.tile([128, MCH, CI], BF16, tag="gTh")
        gT_lo = persist.tile([128, MCH, CI], BF16, tag="gTl")
        for mc in range(MCH):
            tp = ps_cv.tile([128, 128], F32, tag="cv")
            nc.tensor.transpose(tp, g_pool[:, mc * 128:(mc + 1) * 128], ident)
            nc.scalar.copy(out=gT32[:, mc, :], in_=tp)
            nc.scalar.copy(out=gT_hi[:, mc, :], in_=gT32[:, mc, :])
            nc.vector.tensor_tensor(out=gT_lo[:, mc, :], in0=gT32[:, mc, :],
                                    in1=gT_hi[:, mc, :], op=ALU.subtract)

        # ---- attention + normalization + W-conv, per n-tile ----
        y_n = persist.tile([128, N], F32, tag="yn")
        wy = [scratch.tile([128, N], F32, tag="s4", name=f"wy{ch}")
              for ch in range(CCH)]
        rT = persist.tile([128, NTILES * (NT // 128)], F32, tag="rT")
        s1p = persist.tile([128, CCH, NTILES], F32, tag="s1p")
        s2p = persist.tile([128, CCH, NTILES], F32, tag="s2p")

        for it in range(NTILES):
            sl = slice(it * NT, (it + 1) * NT)
            yps = ps_y.tile([128, NT], F32, tag="yps")
            sps = ps_s.tile([1, NT], F32, tag="sps")
            for mc in range(MCH):
                ms = slice(mc * 128, (mc + 1) * 128)
                fps = ps_ft.tile([128, NT], F32, tag="ft")
                nc.tensor.matmul(fps, lhsT=phi_hi[:, ms], rhs=th_hi[:, sl],
                                 start=True, stop=False)
                nc.tensor.matmul(fps, lhsT=phi_hi[:, ms], rhs=th_lo[:, sl],
                                 start=False, stop=False)
                nc.tensor.matmul(fps, lhsT=phi_lo[:, ms], rhs=th_hi[:, sl],
                                 start=False, stop=True)
                ef = efp.tile([128, NT], BF16, tag="ef")
                nc.scalar.activation(out=ef, in_=fps, func=AF.Exp)
                nc.tensor.matmul(yps, lhsT=gT_hi[:, mc, :], rhs=ef,
                                 start=(mc == 0), stop=False)
                nc.tensor.matmul(yps, lhsT=gT_lo[:, mc, :], rhs=ef,
                                 start=False, stop=(mc == MCH - 1))
                nc.tensor.matmul(sps, lhsT=ones_k, rhs=ef,
                                 start=(mc == 0), stop=(mc == MCH - 1))

            # s -> SBUF, bounce via DRAM into (128, NT/128) layout, recip, back
            s_sb = small.tile([1, NT], F32, tag="s1d")
            nc.scalar.copy(out=s_sb, in_=sps)
            nc.sync.dma_start(out=s_dram[:, sl], in_=s_sb)
            f4 = NT // 128
            sl4 = slice(it * f4, (it + 1) * f4)
            sT_t = small.tile([128, f4], F32, tag="sT")
            nc.sync.dma_start(out=sT_t, in_=s_dram[0, sl].rearrange("(p f) -> p f", p=128))
            nc.vector.reciprocal(out=rT[:, sl4], in_=sT_t)
            nc.sync.dma_start(out=r_dram[0, sl].rearrange("(p f) -> p f", p=128),
                              in_=rT[:, sl4])
            r_sb = small.tile([1, NT], F32, tag="r1d")
            nc.sync.dma_start(out=r_sb, in_=r_dram[:, sl])

            # broadcast r across partitions via K=1 matmul, then normalize y
            rbps = ps_rb.tile([128, NT], F32, tag="rb")
            nc.tensor.matmul(rbps, lhsT=ones_p, rhs=r_sb, start=True, stop=True)
            rb_sb = small.tile([128, NT], F32, tag="rb_sb")
            nc.scalar.copy(out=rb_sb, in_=rbps)
            nc.vector.scalar_tensor_tensor(
                out=y_n[:, sl], in0=yps, scalar=1.0, in1=rb_sb,
                op0=ALU.mult, op1=ALU.mult)

            # W conv on normalized y (native fp32); accumulate BN partial stats
            for ch in range(CCH):
                wps = ps_cv.tile([128, NT], F32, tag="cv")
                nc.tensor.matmul(wps, lhsT=ww_sb[:, ch, :], rhs=y_n[:, sl],
                                 start=True, stop=True)
                nc.vector.tensor_scalar(
                    out=wy[ch][:, sl], in0=wps, scalar1=0.0, scalar2=None,
                    op0=ALU.add, op1=ALU.add, accum_out=s1p[:, ch, it:it + 1])
                sqt = efp.tile([128, NT], BF16, tag="sqtrash")
                nc.scalar.activation(
                    out=sqt, in_=wy[ch][:, sl], func=AF.Square,
                    accum_out=s2p[:, ch, it:it + 1])

        # ---- combine partials, AllGather, local sum, finalize ----
        stats_sb = small.tile([128, 2 * CCH], F32, tag="stats")
        for ch in range(CCH):
            nc.vector.tensor_reduce(out=stats_sb[:, 2 * ch:2 * ch + 1],
                                    in_=s1p[:, ch, :], axis=AX.X, op=ALU.add)
            nc.vector.tensor_reduce(out=stats_sb[:, 2 * ch + 1:2 * ch + 2],
                                    in_=s2p[:, ch, :], axis=AX.X, op=ALU.add)
        nc.sync.dma_start(out=stats_in[:, :], in_=stats_sb)
        nc.gpsimd.collective_compute(
            "AllReduce", ALU.add, replica_groups=[list(range(B))],
            ins=[stats_in[:, :]], outs=[stats_out[:, :]])
        stats_g = small.tile([128, 2 * CCH], F32, tag="statsg")
        nc.sync.dma_start(out=stats_g, in_=stats_out[:, :])

        out_sb = small.tile([128, CCH], F32, tag="outsb")
        for ch in range(CCH):
            mean = small.tile([128, 1], F32, tag="fin")
            e2 = small.tile([128, 1], F32, tag="fin")
            m2 = small.tile([128, 1], F32, tag="fin")
            var = small.tile([128, 1], F32, tag="fin")
            nc.vector.tensor_scalar_mul(out=mean, in0=stats_g[:, 2 * ch:2 * ch + 1],
                                        scalar1=INV_CNT)
            nc.vector.tensor_scalar_mul(out=e2, in0=stats_g[:, 2 * ch + 1:2 * ch + 2],
                                        scalar1=INV_CNT)
            nc.scalar.square(out=m2, in_=mean)
            nc.vector.tensor_tensor(out=var, in0=e2, in1=m2, op=ALU.subtract)
            sd = small.tile([128, 1], F32, tag="fin")
            nc.scalar.activation(out=sd, in_=var, func=AF.Sqrt, bias=eps_sb, scale=1.0)
            inv = small.tile([128, 1], F32, tag="fin")
            nc.vector.reciprocal(out=inv, in_=sd)
            scale = small.tile([128, 1], F32, tag="fin")
            nc.vector.tensor_tensor(out=scale, in0=inv, in1=gamma_sb[:, ch:ch + 1],
                                    op=ALU.mult)
            negshift = small.tile([128, 1], F32, tag="fin")
            nc.vector.scalar_tensor_tensor(
                out=negshift, in0=mean, scalar=scale, in1=beta_sb[:, ch:ch + 1],
                op0=ALU.mult, op1=ALU.subtract)
            # z' = wy*scale + x (in place over wy); out = max_n z' - negshift
            nc.vector.scalar_tensor_tensor(
                out=wy[ch][:, :], in0=wy[ch][:, :], scalar=scale, in1=x_sb[ch],
                op0=ALU.mult, op1=ALU.add)
            mx = small.tile([128, 1], F32, tag="fin")
            nc.vector.tensor_reduce(out=mx, in_=wy[ch][:, :], axis=AX.X, op=ALU.max)
            nc.vector.tensor_tensor(out=out_sb[:, ch:ch + 1], in0=mx, in1=negshift,
                                    op=ALU.subtract)
        for ch in range(CCH):
            nc.sync.dma_start(out=out_d[ch, :].rearrange("(p one) -> p one", one=1),
                              in_=out_sb[:, ch:ch + 1])

    nc.compile()
    return nc


_LAST = {}


def kernel(**inputs):
    x = np.ascontiguousarray(inputs["x"], dtype=np.float32)      # (8, 256, 64, 64)
    Wg = np.asarray(inputs["Wg"], dtype=np.float32)
    bg = np.asarray(inputs["bg"], dtype=np.float32)
    Wt = np.asarray(inputs["Wt"], dtype=np.float32)
    bt = np.asarray(inputs["bt"], dtype=np.float32)
    Wp = np.asarray(inputs["Wp"], dtype=np.float32)
    bp = np.asarray(inputs["bp"], dtype=np.float32)
    Ww = np.asarray(inputs["Ww"], dtype=np.float32)
    gamma = np.asarray(inputs["gamma"], dtype=np.float32)
    beta = np.asarray(inputs["beta"], dtype=np.float32)

    if "nc" not in _CACHE:
        _CACHE["nc"] = _build()
    nc = _CACHE["nc"]

    try:
        import ml_dtypes
        bf = ml_dtypes.bfloat16
    except ImportError:
        import jax.numpy as jnp
        bf = jnp.bfloat16

    def split(w):
        hi = np.ascontiguousarray(w.astype(bf))
        lo = np.ascontiguousarray((w - hi.astype(np.float32)).astype(bf))
        return hi, lo

    WtThi, WtTlo = split(np.ascontiguousarray(Wt.T))
    WpThi, WpTlo = split(np.ascontiguousarray(Wp.T))
    WgThi, WgTlo = split(np.ascontiguousarray(Wg.T))

    shared = {
        "WtThi": WtThi, "WtTlo": WtTlo,
        "WpThi": WpThi, "WpTlo": WpTlo,
        "WgThi": WgThi, "WgTlo": WgTlo,
        "WwT": np.ascontiguousarray(Ww.T),
        "bt": np.ascontiguousarray(bt.reshape(CI, 1)),
        "bp": np.ascontiguousarray(bp.reshape(CI, 1)),
        "bg": np.ascontiguousarray(bg.reshape(CI, 1)),
        "gamma": np.ascontiguousarray(gamma.reshape(CCH, 128).T),
        "beta": np.ascontiguousarray(beta.reshape(CCH, 128).T),
    }
    in_maps = [dict(shared, x=np.ascontiguousarray(x[b].reshape(C, N)))
               for b in range(B)]
    import os
    trace = bool(int(os.environ.get("KERNEL_TRACE", "0")))
    res = run_bass_kernel_spmd(nc, in_maps, core_ids=list(range(B)), trace=trace)
    _LAST["res"] = res
    out = np.stack([np.asarray(res.results[b]["out"]).reshape(C) for b in range(B)])
    return out.reshape(B, C, 1, 1).astype(np.float32)


if __name__ == "__main__":
    pass
